# revision 14
# baseline (speedup 1.0000x reference)
"""KGram MLP seq model on 8 TRN2 NeuronCores.

Reference computation (per token t, batch b):
    ctx   = last K=3 token ids before t (left-padded with id 0)
    x     = concat(emb[ctx])                       # (3*1024,)
    h     = silu(x @ W1 + b1)                      # (1024,)
    logit = h @ W2 + b2                            # (32000,)

Sharding: data-parallel over the sequence axis. Core c handles seq
positions [c*256, (c+1)*256) for all 4 batch columns (1024 tokens/core).

Per-core device kernel:
  1. indirect-DMA gather of the (halo-extended) token embeddings, bf16
  2. PE transpose -> ET[b][e] = emb chunk e, embed-dim on partitions
  3. layer 1: hT = silu(W1.T @ x^T + b1) via 24 accumulating matmuls per
     (batch, hidden-chunk) using 3 shifted views of ET (no concat needed)
  4. layer 2: stream W2 in 512-wide vocab chunks; out = hT.T @ W2 + b2,
     PSUM-accumulated over the 8 hidden chunks; DVE adds broadcast b2
     while copying PSUM->SBUF; DMA to DRAM.

Matmuls run in bf16 (fp32 PSUM accumulation); weights/embeddings are
cast to bf16 on host. Output stays fp32.
"""

import numpy as np
import ml_dtypes

import concourse.bass as bass
import concourse.mybir as mybir
import concourse.tile as tile
from concourse import bacc
from concourse.bass_utils import run_bass_kernel_spmd
from concourse.masks import make_identity

BF16 = ml_dtypes.bfloat16

# Full-size problem constants (hardcoded per spec).
S, B, V, E, H, KC = 2048, 4, 32000, 1024, 1024, 3
N_CORES = 8
S_SH = S // N_CORES          # 256 seq positions per core
NV = 512                     # vocab chunk width (one PSUM bank)


def build_nc(s_sh=S_SH, b=B, v=V, e=E, h=H, vt=V):
    """Build the per-core Bass program. vt = embedding-table rows."""
    dt = mybir.dt
    ec, hc = e // 128, h // 128
    tokl = s_sh * b              # local tokens
    tt = tokl // 128             # 128-token output tiles
    nj = (s_sh + KC + 127) // 128  # 128-row gather tiles per batch
    etw = nj * 128               # ET free width (includes halo + pad)
    nvc = (v + NV - 1) // NV     # vocab chunks
    assert s_sh % 128 == 0 and e % 128 == 0 and h % 128 == 0

    nc = bacc.Bacc("TRN2", target_bir_lowering=False, debug=False,
                   num_devices=1)

    idx_d = nc.dram_tensor("idx", [128, b * nj], dt.int32,
                           kind="ExternalInput")
    emb_d = nc.dram_tensor("emb", [vt, e], dt.bfloat16, kind="ExternalInput")
    w1_d = nc.dram_tensor("w1", [KC * e, h], dt.bfloat16,
                          kind="ExternalInput")
    # w2 host-relaid as [128, hc, v]: [p, k, vv] = W2[k*128+p, vv] so one
    # DMA fetches a whole vocab chunk (all hidden k-tiles).
    w2_d = nc.dram_tensor("w2", [128, hc, v], dt.bfloat16,
                          kind="ExternalInput")
    b1_d = nc.dram_tensor("b1", [hc, 128, 1], dt.float32,
                          kind="ExternalInput")
    b2_d = nc.dram_tensor("b2", [1, v], dt.float32, kind="ExternalInput")
    out_d = nc.dram_tensor("out", [tokl, v], dt.float32,
                           kind="ExternalOutput")

    with tile.TileContext(nc) as tc:
        with (
            tc.tile_pool(name="const", bufs=1) as constp,
            tc.tile_pool(name="stage", bufs=3) as stagep,
            tc.tile_pool(name="w2p", bufs=3) as w2p,
            tc.tile_pool(name="outp", bufs=8) as outp,
            tc.tile_pool(name="b2p", bufs=2) as b2p,
            tc.tile_pool(name="ps", bufs=6, space="PSUM") as psp,
        ):
            ident = constp.tile([128, 128], dt.bfloat16, tag="ident")
            make_identity(nc, ident[:])

            # Gather indices first: one DMA, then the 12 indirect gathers
            # launch immediately (they gate the first PE transposes).
            idxt = constp.tile([128, b * nj], dt.int32, tag="idxt")
            nc.scalar.dma_start(idxt[:], idx_d[:])
            egs = []
            for bb in range(b):
                for j in range(nj):
                    eg = stagep.tile([128, e], dt.bfloat16, tag="eg",
                                     bufs=2 * b)
                    nc.gpsimd.indirect_dma_start(
                        out=eg[:], out_offset=None,
                        in_=emb_d[:, :],
                        in_offset=bass.IndirectOffsetOnAxis(
                            ap=idxt[:, bb * nj + j:bb * nj + j + 1], axis=0),
                    )
                    egs.append(eg)

            # W1 resident in SBUF as 24 (128, H) bf16 tiles (k-chunk major).
            w1sb = []
            for k in range(KC * ec):
                t = constp.tile([128, h], dt.bfloat16, tag=f"w1_{k}")
                nc.sync.dma_start(t[:], w1_d[k * 128:(k + 1) * 128, :])
                w1sb.append(t)
            b1sb = []
            for m in range(hc):
                t = constp.tile([128, 1], dt.float32, tag=f"b1_{m}")
                nc.sync.dma_start(t[:], b1_d[m])
                b1sb.append(t)

            # Transpose gathers to ET[b][e] (embed-dim on partitions,
            # halo-extended positions on free dim).
            ET = [[constp.tile([128, etw], dt.bfloat16, tag=f"et_{bb}_{ee}", name=f"et_{bb}_{ee}")
                   for ee in range(ec)] for bb in range(b)]
            for bb in range(b):
                for j in range(nj):
                    eg = egs[bb * nj + j]
                    for ee in range(ec):
                        pt = psp.tile([128, 128], dt.bfloat16, tag="pst",
                                      bufs=2)
                        nc.tensor.transpose(pt[:],
                                            eg[:, ee * 128:(ee + 1) * 128],
                                            ident[:])
                        nc.vector.tensor_copy(
                            ET[bb][ee][:, j * 128:(j + 1) * 128],
                            pt[:])

            # Layer 1: hT[m] (hidden-chunk m on partitions, local token on
            # free dim, bf16).  Local token index = b*s_sh + s_local.
            hT = [constp.tile([128, tokl], dt.bfloat16, tag=f"ht_{m}", name=f"ht_{m}")
                  for m in range(hc)]
            for bb in range(b):
                for m in range(hc):
                    hp = psp.tile([128, NV], dt.float32, tag="ps")
                    for s in range(KC):
                        for ee in range(ec):
                            nc.tensor.matmul(
                                hp[:, :s_sh],
                                lhsT=w1sb[s * ec + ee][:, m * 128:(m + 1) * 128],
                                rhs=ET[bb][ee][:, s:s + s_sh],
                                start=(s == 0 and ee == 0),
                                stop=(s == KC - 1 and ee == ec - 1))
                    pre = stagep.tile([128, s_sh], dt.float32, tag="pre",
                                      bufs=3)
                    nc.vector.tensor_add(
                        pre[:], hp[:, :s_sh],
                        b1sb[m][:].to_broadcast((128, s_sh)))
                    sg = stagep.tile([128, s_sh], dt.float32, tag="sg",
                                     bufs=3)
                    nc.scalar.activation(
                        sg[:], pre[:], mybir.ActivationFunctionType.Sigmoid)
                    nc.vector.tensor_mul(
                        hT[m][:, bb * s_sh:(bb + 1) * s_sh], pre[:], sg[:])

            # Layer 2: stream W2 vocab chunks; PSUM-accumulate over hidden.
            for vc in range(nvc):
                v0 = vc * NV
                nv = min(NV, v - v0)
                w2t = w2p.tile([128, hc * NV], dt.bfloat16, tag="w2", bufs=3)
                nc.sync.dma_start(w2t[:, :hc * nv], w2_d[:, :, v0:v0 + nv])
                b2t = b2p.tile([128, NV], dt.float32, tag="b2")
                nc.scalar.dma_start(b2t[:, :nv],
                                  b2_d[0:1, v0:v0 + nv].to_broadcast((128, nv)))
                for t in range(tt):
                    op = psp.tile([128, NV], dt.float32, tag="ps")
                    for k in range(hc):
                        nc.tensor.matmul(op[:, :nv],
                                         lhsT=hT[k][:, t * 128:(t + 1) * 128],
                                         rhs=w2t[:, k * nv:(k + 1) * nv],
                                         start=(k == 0), stop=(k == hc - 1))
                    ot = outp.tile([128, NV], dt.float32, tag="ot")
                    nc.vector.tensor_add(ot[:, :nv], op[:, :nv], b2t[:, :nv])
                    nc.scalar.dma_start(out_d[t * 128:(t + 1) * 128, v0:v0 + nv],
                                        ot[:, :nv])

    nc.compile()
    return nc


def make_idx(tokens_col, s0, s_sh):
    """Gather indices for one (core, batch) column: positions
    [s0-3, s0-3+nj*128) of the 3-left-padded sequence, clamped.
    Returns (nj*128,) int32."""
    nj = (s_sh + KC + 127) // 128
    padded = np.concatenate([np.zeros(KC, np.int64),
                             tokens_col.astype(np.int64)])
    pos = np.arange(s0, s0 + nj * 128)
    pos = np.clip(pos, 0, len(padded) - 1)
    return padded[pos].astype(np.int32)


_NC_CACHE = {}
TRACE = False          # set by test harness to capture an NTFF profile
LAST_RESULTS = None    # BassKernelResults of the most recent kernel() call


def kernel(tokens_seq, emb, W1, b1, W2, b2):
    tokens_seq = np.asarray(tokens_seq)
    emb = np.asarray(emb, dtype=np.float32)
    W1 = np.asarray(W1, dtype=np.float32)
    W2 = np.asarray(W2, dtype=np.float32)
    b1 = np.asarray(b1, dtype=np.float32)
    b2 = np.asarray(b2, dtype=np.float32)

    if "full" not in _NC_CACHE:
        _NC_CACHE["full"] = build_nc()
    nc = _NC_CACHE["full"]

    emb_bf = emb.astype(BF16)
    w1_bf = W1.astype(BF16)
    # [p, k, v] = W2[k*128+p, v] so one DMA fetches a whole vocab chunk
    w2_bf = np.ascontiguousarray(
        W2.astype(BF16).reshape(H // 128, 128, V).transpose(1, 0, 2))
    b1_r = np.ascontiguousarray(b1.reshape(H // 128, 128, 1))
    b2_r = np.ascontiguousarray(b2.reshape(1, V))

    in_maps = []
    for c in range(N_CORES):
        s0 = c * S_SH
        idx = np.stack([make_idx(tokens_seq[:, bb], s0, S_SH)
                        for bb in range(B)], axis=0)  # (B*NJ... , 128)
        idx = np.ascontiguousarray(
            idx.reshape(-1, 128).T)                   # (128, B*nj)
        in_maps.append({
            "idx": idx,
            "emb": emb_bf, "w1": w1_bf, "w2": w2_bf,
            "b1": b1_r, "b2": b2_r,
        })

    res = run_bass_kernel_spmd(nc, in_maps, core_ids=list(range(N_CORES)),
                               trace=TRACE)
    global LAST_RESULTS
    LAST_RESULTS = res

    out = np.empty((S, B, V), dtype=np.float32)
    for c in range(N_CORES):
        o = res.results[c]["out"].reshape(B, S_SH, V)
        out[c * S_SH:(c + 1) * S_SH] = o.transpose(1, 0, 2)
    return out


# revision 16
# speedup vs baseline: 1.0022x; 1.0022x over previous
"""KGram MLP seq model on 8 TRN2 NeuronCores.

Reference computation (per token t, batch b):
    ctx   = last K=3 token ids before t (left-padded with id 0)
    x     = concat(emb[ctx])                       # (3*1024,)
    h     = silu(x @ W1 + b1)                      # (1024,)
    logit = h @ W2 + b2                            # (32000,)

Sharding: data-parallel over the sequence axis. Core c handles seq
positions [c*256, (c+1)*256) for all 4 batch columns (1024 tokens/core).

Per-core device kernel:
  1. indirect-DMA gather of the (halo-extended) token embeddings, bf16
  2. PE transpose -> ET[b][e] = emb chunk e, embed-dim on partitions
  3. layer 1: hT = silu(W1.T @ x^T + b1) via 24 accumulating matmuls per
     (batch, hidden-chunk) using 3 shifted views of ET (no concat needed)
  4. layer 2: stream W2 in 512-wide vocab chunks; out = hT.T @ W2 + b2,
     PSUM-accumulated over the 8 hidden chunks; DVE adds broadcast b2
     while copying PSUM->SBUF; DMA to DRAM.

Matmuls run in bf16 (fp32 PSUM accumulation); weights/embeddings are
cast to bf16 on host. Output stays fp32.
"""

import numpy as np
import ml_dtypes

import concourse.bass as bass
import concourse.mybir as mybir
import concourse.tile as tile
from concourse import bacc
from concourse.bass_utils import run_bass_kernel_spmd
from concourse.masks import make_identity

BF16 = ml_dtypes.bfloat16

# Full-size problem constants (hardcoded per spec).
S, B, V, E, H, KC = 2048, 4, 32000, 1024, 1024, 3
N_CORES = 8
S_SH = S // N_CORES          # 256 seq positions per core
NV = 512                     # vocab chunk width (one PSUM bank)


def build_nc(s_sh=S_SH, b=B, v=V, e=E, h=H, vt=V):
    """Build the per-core Bass program. vt = embedding-table rows."""
    dt = mybir.dt
    ec, hc = e // 128, h // 128
    tokl = s_sh * b              # local tokens
    tt = tokl // 128             # 128-token output tiles
    nj = (s_sh + KC + 127) // 128  # 128-row gather tiles per batch
    etw = nj * 128               # ET free width (includes halo + pad)
    nvc = (v + NV - 1) // NV     # vocab chunks
    assert s_sh % 128 == 0 and e % 128 == 0 and h % 128 == 0

    nc = bacc.Bacc("TRN2", target_bir_lowering=False, debug=False,
                   num_devices=1)

    idx_d = nc.dram_tensor("idx", [128, b * nj], dt.int32,
                           kind="ExternalInput")
    emb_d = nc.dram_tensor("emb", [vt, e], dt.bfloat16, kind="ExternalInput")
    w1_d = nc.dram_tensor("w1", [KC * e, h], dt.bfloat16,
                          kind="ExternalInput")
    # w2 host-relaid as [128, hc, v]: [p, k, vv] = W2[k*128+p, vv] so one
    # DMA fetches a whole vocab chunk (all hidden k-tiles).
    w2_d = nc.dram_tensor("w2", [128, hc, v], dt.bfloat16,
                          kind="ExternalInput")
    b1_d = nc.dram_tensor("b1", [hc, 128, 1], dt.float32,
                          kind="ExternalInput")
    b2_d = nc.dram_tensor("b2", [1, v], dt.float32, kind="ExternalInput")
    out_d = nc.dram_tensor("out", [tokl, v], dt.float32,
                           kind="ExternalOutput")

    with tile.TileContext(nc) as tc:
        with (
            tc.tile_pool(name="const", bufs=1) as constp,
            tc.tile_pool(name="stage", bufs=3) as stagep,
            tc.tile_pool(name="w2p", bufs=3) as w2p,
            tc.tile_pool(name="outp", bufs=8) as outp,
            tc.tile_pool(name="b2p", bufs=2) as b2p,
            tc.tile_pool(name="ps", bufs=6, space="PSUM") as psp,
        ):
            # Gather indices first: one DMA, then the 12 indirect gathers
            # launch immediately (they gate the first PE transposes).
            idxt = constp.tile([128, b * nj], dt.int32, tag="idxt")
            nc.scalar.dma_start(idxt[:], idx_d[:])
            egs = []
            for bb in range(b):
                for j in range(nj):
                    eg = stagep.tile([128, e], dt.bfloat16, tag="eg",
                                     bufs=2 * b)
                    nc.gpsimd.indirect_dma_start(
                        out=eg[:], out_offset=None,
                        in_=emb_d[:, :],
                        in_offset=bass.IndirectOffsetOnAxis(
                            ap=idxt[:, bb * nj + j:bb * nj + j + 1], axis=0),
                    )
                    egs.append(eg)

            ident = constp.tile([128, 128], dt.bfloat16, tag="ident")
            make_identity(nc, ident[:])

            # PE warm-up: dummy transposes while the first gathers are in
            # flight — ramps the HAM clock gate (1.2->2.4 GHz takes ~3.4us
            # of activity) and warms the tensor I$ during otherwise-idle
            # startup. Results are never read.
            for _wu in range(40):
                wut = psp.tile([128, 128], dt.bfloat16, tag="pst", bufs=2,
                               name="wut")
                nc.tensor.transpose(wut[:], ident[:], ident[:])

            # W1 resident in SBUF as 24 (128, H) bf16 tiles (k-chunk major).
            w1sb = []
            for k in range(KC * ec):
                t = constp.tile([128, h], dt.bfloat16, tag=f"w1_{k}")
                nc.sync.dma_start(t[:], w1_d[k * 128:(k + 1) * 128, :])
                w1sb.append(t)
            b1sb = []
            for m in range(hc):
                t = constp.tile([128, 1], dt.float32, tag=f"b1_{m}")
                nc.sync.dma_start(t[:], b1_d[m])
                b1sb.append(t)

            # Transpose gathers to ET[b][e] (embed-dim on partitions,
            # halo-extended positions on free dim).
            ET = [[constp.tile([128, etw], dt.bfloat16, tag=f"et_{bb}_{ee}", name=f"et_{bb}_{ee}")
                   for ee in range(ec)] for bb in range(b)]
            for bb in range(b):
                for j in range(nj):
                    eg = egs[bb * nj + j]
                    for ee in range(ec):
                        pt = psp.tile([128, 128], dt.bfloat16, tag="pst",
                                      bufs=2)
                        nc.tensor.transpose(pt[:],
                                            eg[:, ee * 128:(ee + 1) * 128],
                                            ident[:])
                        nc.vector.tensor_copy(
                            ET[bb][ee][:, j * 128:(j + 1) * 128],
                            pt[:])

            # Layer 1: hT[m] (hidden-chunk m on partitions, local token on
            # free dim, bf16).  Local token index = b*s_sh + s_local.
            hT = [constp.tile([128, tokl], dt.bfloat16, tag=f"ht_{m}", name=f"ht_{m}")
                  for m in range(hc)]
            for bb in range(b):
                for m in range(hc):
                    hp = psp.tile([128, NV], dt.float32, tag="ps")
                    for s in range(KC):
                        for ee in range(ec):
                            nc.tensor.matmul(
                                hp[:, :s_sh],
                                lhsT=w1sb[s * ec + ee][:, m * 128:(m + 1) * 128],
                                rhs=ET[bb][ee][:, s:s + s_sh],
                                start=(s == 0 and ee == 0),
                                stop=(s == KC - 1 and ee == ec - 1))
                    pre = stagep.tile([128, s_sh], dt.float32, tag="pre",
                                      bufs=3)
                    nc.vector.tensor_add(
                        pre[:], hp[:, :s_sh],
                        b1sb[m][:].to_broadcast((128, s_sh)))
                    sg = stagep.tile([128, s_sh], dt.float32, tag="sg",
                                     bufs=3)
                    nc.scalar.activation(
                        sg[:], pre[:], mybir.ActivationFunctionType.Sigmoid)
                    nc.vector.tensor_mul(
                        hT[m][:, bb * s_sh:(bb + 1) * s_sh], pre[:], sg[:])

            # Layer 2: stream W2 vocab chunks; PSUM-accumulate over hidden.
            for vc in range(nvc):
                v0 = vc * NV
                nv = min(NV, v - v0)
                w2t = w2p.tile([128, hc * NV], dt.bfloat16, tag="w2", bufs=3)
                nc.sync.dma_start(w2t[:, :hc * nv], w2_d[:, :, v0:v0 + nv])
                b2t = b2p.tile([128, NV], dt.float32, tag="b2")
                nc.scalar.dma_start(b2t[:, :nv],
                                  b2_d[0:1, v0:v0 + nv].to_broadcast((128, nv)))
                for t in range(tt):
                    op = psp.tile([128, NV], dt.float32, tag="ps")
                    for k in range(hc):
                        nc.tensor.matmul(op[:, :nv],
                                         lhsT=hT[k][:, t * 128:(t + 1) * 128],
                                         rhs=w2t[:, k * nv:(k + 1) * nv],
                                         start=(k == 0), stop=(k == hc - 1))
                    ot = outp.tile([128, NV], dt.float32, tag="ot")
                    nc.vector.tensor_add(ot[:, :nv], op[:, :nv], b2t[:, :nv])
                    nc.scalar.dma_start(out_d[t * 128:(t + 1) * 128, v0:v0 + nv],
                                        ot[:, :nv])

    nc.compile()
    return nc


def make_idx(tokens_col, s0, s_sh):
    """Gather indices for one (core, batch) column: positions
    [s0-3, s0-3+nj*128) of the 3-left-padded sequence, clamped.
    Returns (nj*128,) int32."""
    nj = (s_sh + KC + 127) // 128
    padded = np.concatenate([np.zeros(KC, np.int64),
                             tokens_col.astype(np.int64)])
    pos = np.arange(s0, s0 + nj * 128)
    pos = np.clip(pos, 0, len(padded) - 1)
    return padded[pos].astype(np.int32)


_NC_CACHE = {}
TRACE = False          # set by test harness to capture an NTFF profile
LAST_RESULTS = None    # BassKernelResults of the most recent kernel() call


def kernel(tokens_seq, emb, W1, b1, W2, b2):
    tokens_seq = np.asarray(tokens_seq)
    emb = np.asarray(emb, dtype=np.float32)
    W1 = np.asarray(W1, dtype=np.float32)
    W2 = np.asarray(W2, dtype=np.float32)
    b1 = np.asarray(b1, dtype=np.float32)
    b2 = np.asarray(b2, dtype=np.float32)

    if "full" not in _NC_CACHE:
        _NC_CACHE["full"] = build_nc()
    nc = _NC_CACHE["full"]

    emb_bf = emb.astype(BF16)
    w1_bf = W1.astype(BF16)
    # [p, k, v] = W2[k*128+p, v] so one DMA fetches a whole vocab chunk
    w2_bf = np.ascontiguousarray(
        W2.astype(BF16).reshape(H // 128, 128, V).transpose(1, 0, 2))
    b1_r = np.ascontiguousarray(b1.reshape(H // 128, 128, 1))
    b2_r = np.ascontiguousarray(b2.reshape(1, V))

    in_maps = []
    for c in range(N_CORES):
        s0 = c * S_SH
        idx = np.stack([make_idx(tokens_seq[:, bb], s0, S_SH)
                        for bb in range(B)], axis=0)  # (B*NJ... , 128)
        idx = np.ascontiguousarray(
            idx.reshape(-1, 128).T)                   # (128, B*nj)
        in_maps.append({
            "idx": idx,
            "emb": emb_bf, "w1": w1_bf, "w2": w2_bf,
            "b1": b1_r, "b2": b2_r,
        })

    res = run_bass_kernel_spmd(nc, in_maps, core_ids=list(range(N_CORES)),
                               trace=TRACE)
    global LAST_RESULTS
    LAST_RESULTS = res

    out = np.empty((S, B, V), dtype=np.float32)
    for c in range(N_CORES):
        o = res.results[c]["out"].reshape(B, S_SH, V)
        out[c * S_SH:(c + 1) * S_SH] = o.transpose(1, 0, 2)
    return out
